# revision 17
# baseline (speedup 1.0000x reference)
"""Fused LayerNorm + causal multi-head attention (with additive bias) + out-proj
for Trainium2, SPMD over 8 NeuronCores.

Sharding: tensor-parallel over heads. 16 heads / 8 cores = 2 heads per core.
Each core computes LN(x) (replicated), the qkv projection restricted to its
2 heads' columns, causal softmax attention with its heads' bias slices, and a
partial output projection (its heads' rows of w_out). Host sums the 8 partial
outputs (the TP all-reduce, done on gather).

v3 design notes:
 - The additive attention bias enters MULTIPLICATIVELY after exp:
   exp(S + bias) = exp(S) * exp(bias). The host ships ebias = exp(bias) with
   zeros above the causal diagonal (bf16); the multiply runs on the DVE.
   This removes the per-(b,h,i,j) identity-matmul bias adds from the PE.
 - Scores for both heads of a batch go into one [128, 1024] PSUM tile: the
   two score matmuls row-group-pack (h0 rows 0-63, h1 rows 64-127) and one
   exp activation covers both heads.
 - The causal structure lets attention i-tile t start once qkv is done for
   token slice t: qkv for slice nt+1 is emitted interleaved inside attention
   i-tile t's j-loop, so the PE never drains (HAM stays at K=8/8) and the
   exp/multiply engines overlap qkv matmuls instead of idling.
 - All LayerNorm stats (the only Sqrt user) complete before the first Exp,
   so the ACT table set switches exactly once (sqrt set -> exp set).
 - V is transposed to natural layout by the DMA crossbar (dma_start_transpose)
   instead of PE transposes: no PSUM bank, no PE time.
 - The softmax denominator comes from a [128,1]-ones stationary matmul into
   partition 64 of the same PSUM bank as P@V (col group 2, concurrent with
   the P@V matmul on col groups 0-1).
 - LN stats matmuls col-group-pack via tile_position; their accumulators
   borrow PSUM from the pss/pso pools (PSUM is exactly full: 2x[128,1024]
   score tiles + 4x[65,512] output accumulators).
 - The out-projection runs per i-tile inside the attention loop; y partials
   ship as fp16.
 - The first i-tile's ebias tiles are DMA'd before the big x load so the
   attention start is not FIFO-blocked behind 16 MB of x.
"""

import numpy as np
import ml_dtypes
from contextlib import ExitStack

import concourse.bass as bass
import concourse.tile as tile
from concourse import bacc, mybir
from concourse.bass_utils import run_bass_kernel_spmd

F32 = mybir.dt.float32
F16 = mybir.dt.float16
BF16 = mybir.dt.bfloat16
AL = mybir.AluOpType
AF = mybir.ActivationFunctionType

N_CORES = 8
B = 2            # batch
N = 2048         # tokens
D = 1024         # model dim
H = 16           # total heads
HL = 2           # heads per core
DH = 64          # head dim
COLS = 3 * HL * DH   # 384 qkv columns per core
KS = D // 128    # 8 contraction slabs
TT = N // 128    # 16 token tiles
IT = N // 512    # 4 i-tiles (query tiles of 512)
SCALE = DH ** -0.5
LN_EPS = 1e-5


def build_program():
    nc = bacc.Bacc("TRN2", target_bir_lowering=False, debug=False)

    xT_in = nc.dram_tensor("xT", [B, D, N], BF16, kind="ExternalInput")
    ebias_in = nc.dram_tensor("ebias", [HL, N, N], BF16, kind="ExternalInput")
    wqkv_in = nc.dram_tensor("wqkv", [128, KS, COLS], F32, kind="ExternalInput")
    wout_in = nc.dram_tensor("wout", [HL * DH, D], F32, kind="ExternalInput")
    gamma_in = nc.dram_tensor("gamma", [128, KS], F32, kind="ExternalInput")
    beta_in = nc.dram_tensor("beta", [128, KS], F32, kind="ExternalInput")
    y_out = nc.dram_tensor("y", [B, N, D], F16, kind="ExternalOutput")

    MM = dict(skip_group_check=True)

    with tile.TileContext(nc) as tc, ExitStack() as ctx:
        # ---- persistent sbuf ----
        pers = ctx.enter_context(tc.tile_pool(name="pers", bufs=1))
        qT = [pers.tile([128, N], BF16, tag=f"qT{b}", name=f"qT{b}") for b in range(B)]
        kT = [pers.tile([128, N], BF16, tag=f"kT{b}", name=f"kT{b}") for b in range(B)]
        vT = [pers.tile([128, N], BF16, tag=f"vT{b}", name=f"vT{b}") for b in range(B)]
        # V natural: per key-tile [128 keys, 128] = (h0 64 | h1 64)
        vA = [pers.tile([128, TT, 128], BF16, tag=f"vA{b}", name=f"vA{b}") for b in range(B)]
        # attention output per (b, i-tile) so the out-proj can start per tile
        oT = [[pers.tile([128, 512], BF16, tag=f"oT{b}_{t}", name=f"oT{b}_{t}") for t in range(IT)]
              for b in range(B)]
        onesd = pers.tile([128, 1], BF16, tag="onesd")    # 1/D for stats matmuls
        nc.vector.memset(onesd[:], 1.0 / D)
        ones1 = pers.tile([128, 1], BF16, tag="ones1")    # 1.0 for l row-sums
        nc.vector.memset(ones1[:], 1.0)
        epsc = pers.tile([128, 1], F32, tag="epsc")
        nc.vector.memset(epsc[:], LN_EPS)
        zeroc = pers.tile([128, 1], F32, tag="zeroc")
        nc.vector.memset(zeroc[:], 0.0)

        # ---- weights prep ----
        prep_pool = tc.tile_pool(name="prep", bufs=1)
        prep = prep_pool.__enter__()
        wq = prep.tile([128, KS, COLS], F32, tag="wq")
        nc.sync.dma_start(wq[:], wqkv_in.ap())
        # fold the attention scale into the q columns
        nc.vector.tensor_scalar_mul(wq[:, :, 0:128], wq[:, :, 0:128], SCALE)
        gam = prep.tile([128, KS], F32, tag="gam")
        nc.sync.dma_start(gam[:], gamma_in.ap())
        bet = prep.tile([128, KS], F32, tag="bet")
        nc.sync.dma_start(bet[:], beta_in.ap())
        wob = pers.tile([128, D], BF16, tag="wob")
        wof = prep.tile([128, D], F32, tag="wof")
        nc.sync.dma_start(wof[:], wout_in.ap())
        nc.vector.tensor_copy(wob[:], wof[:])

        # gamma-scaled bf16 qkv weights + raw bf16 (for beta @ w)
        wqb = pers.tile([128, KS, COLS], BF16, tag="wqb")
        wrb = prep.tile([128, KS, COLS], BF16, tag="wrb")
        betb = prep.tile([128, KS], BF16, tag="betb")
        nc.vector.tensor_copy(betb[:], bet[:])
        for k in range(KS):
            nc.vector.tensor_scalar_mul(wqb[:, k], wq[:, k], gam[:, k:k + 1])
            nc.vector.tensor_copy(wrb[:, k], wq[:, k])

        # UW33: row0 = -colsum(W') (mu correction), row32 = beta @ W,
        # rows 1-31 zero (engine partition bases must be 32-aligned, so the
        # rank-2 LN correction runs as a 33-row contraction with zero rows)
        UW33 = pers.tile([33, COLS], BF16, tag="UW33")
        nc.vector.memset(UW33[:], 0.0)
        with tc.tile_pool(name="psmall", bufs=2, space="PSUM") as psmall:
            bw_ps = psmall.tile([1, COLS], F32)
            for k in range(KS):
                nc.tensor.matmul(bw_ps[:], betb[:, k:k + 1], wrb[:, k],
                                 start=(k == 0), stop=(k == KS - 1), **MM)
            nc.vector.tensor_copy(UW33[32:33, :], bw_ps[:])
            u_ps = psmall.tile([1, COLS], F32)
            for k in range(KS):
                nc.tensor.matmul(u_ps[:], ones1[:], wqb[:, k],
                                 start=(k == 0), stop=(k == KS - 1), **MM)
            nc.vector.tensor_scalar_mul(UW33[0:1, :], u_ps[:], -1.0)
        prep_pool.__exit__(None, None, None)

        # ---- pools for the whole run ----
        rows = ctx.enter_context(tc.tile_pool(name="rows", bufs=1))
        xpool = ctx.enter_context(tc.tile_pool(name="xT", bufs=2 * KS))
        x2pool = ctx.enter_context(tc.tile_pool(name="x2", bufs=2))
        rbc = ctx.enter_context(tc.tile_pool(name="rbc", bufs=2))
        bias_pool = ctx.enter_context(tc.tile_pool(name="bias", bufs=5))
        exp_pool = ctx.enter_context(tc.tile_pool(name="pexp", bufs=4))
        lnrm = ctx.enter_context(tc.tile_pool(name="lnrm", bufs=2))
        ysb = ctx.enter_context(tc.tile_pool(name="ysb", bufs=2))
        pss_pool = ctx.enter_context(tc.tile_pool(name="pss", bufs=2, space="PSUM"))
        pso_pool = ctx.enter_context(tc.tile_pool(name="pso", bufs=1, space="PSUM"))

        # hoist the first i-tile's ebias loads ahead of the big x DMA (the
        # DMA ring is FIFO; without this the attention start waits ~50us)
        bias_t0 = []
        for j in range(4):
            bt = bias_pool.tile([128, 1024], BF16, tag="bt", name=f"bt0_{j}")
            jt = slice(j * 128, (j + 1) * 128)
            nc.sync.dma_start(
                bt[:].rearrange("p (h i) -> p h i", h=2),
                ebias_in.ap()[0:2, jt, 0:512].rearrange("h j i -> j h i"))
            bias_t0.append(bt)

        # per-batch LN row data: row0 = mu, row32 = sd (for the rank-2 matmul)
        SSD = [pers.tile([33, N], BF16, tag=f"SSD{b}", name=f"SSD{b}") for b in range(B)]
        for b in range(B):
            nc.vector.memset(SSD[b][:], 0.0)
        rsig_bc = [rbc.tile([128, N], BF16, tag=f"rsbc{b}", name=f"rsbc{b}")
                   for b in range(B)]

        # ---- phase S: x load + squares + LN stats + chains (all Sqrt here) ----
        # stats accumulators borrow PSUM: b0 packs 4 i-slices into one pss
        # tile (partition x column quadrants); b1 uses the four pso slots.
        STt = {(b, p): pso_pool.tile([97, 512], F32, tag=f"pso{b}{p}",
                                     name=f"ST{b}{p}") for b in range(B)
               for p in range(2)}

        def st_regions(b, nt):
            st = STt[(b, nt // 2)]
            roff = 64 * (nt % 2)
            return (st[roff:roff + 1, :], (0, roff),
                    st[roff + 32:roff + 33, :], (0, roff + 32))

        xTb = {}
        for k in range(KS):
            for b in range(B):
                xk = xpool.tile([128, N], BF16, tag="xk", name=f"xk{b}_{k}")
                nc.sync.dma_start(xk[:], xT_in.ap()[b, k * 128:(k + 1) * 128, :])
                xTb[b, k] = xk
                x2 = x2pool.tile([128, N], BF16, tag="x2")
                nc.scalar.activation(x2[:], xk[:], AF.Square, bias=zeroc[:])
                for nt in range(IT):
                    sl = slice(nt * 512, (nt + 1) * 512)
                    mu_out, mu_tp, x2_out, x2_tp = st_regions(b, nt)
                    nc.tensor.matmul(mu_out, onesd[:], xk[:, sl],
                                     start=(k == 0), stop=(k == KS - 1),
                                     tile_position=mu_tp, **MM)
                    nc.tensor.matmul(x2_out, onesd[:], x2[:, sl],
                                     start=(k == 0), stop=(k == KS - 1),
                                     tile_position=x2_tp, **MM)
        # LN row chains
        for b in range(B):
            for nt in range(IT):
                sl = slice(nt * 512, (nt + 1) * 512)
                STs = rows.tile([33, 512], F32, tag="STs", bufs=2)
                roff = 64 * (nt % 2)
                nc.vector.tensor_copy(
                    STs[:], STt[(b, nt // 2)][roff:roff + 33, :])
                mu_r = STs[0:1, :]
                x2_r = STs[32:33, :]
                mu2t = rows.tile([33, 512], F32, tag="mu2t", bufs=2)
                nc.vector.tensor_tensor(mu2t[32:33, :], mu_r, mu_r, op=AL.mult)
                var = rows.tile([1, 512], F32, tag="var", bufs=1)
                nc.vector.tensor_tensor(var[:], x2_r, mu2t[32:33, :], op=AL.subtract)
                sd_r = rows.tile([1, 512], F32, tag="sd_r", bufs=1)
                nc.scalar.activation(sd_r[:], var[:], AF.Sqrt, bias=epsc[0:1, :])
                rsig_r = rows.tile([1, 512], F32, tag="rsig_r", bufs=1)
                nc.vector.reciprocal_approx_fast(rsig_r[:], sd_r[:])
                nc.vector.tensor_copy(SSD[b][0:1, sl], mu_r)
                nc.vector.tensor_copy(SSD[b][32:33, sl], sd_r[:])
                rsb = rows.tile([1, 512], BF16, tag="rsb", bufs=1)
                nc.vector.tensor_copy(rsb[:], rsig_r[:])
                nc.gpsimd.partition_broadcast(rsig_bc[b][:, sl], rsb[:],
                                              channels=128)

        # ---- qkv emission helpers ----
        dsts = (qT, kT, vT)

        def emit_qkv_b(b, nt):
            """qkv projections for (batch b, token slice nt): blocks 0+1 pack
            into one [128,1024] pss tile, block 2 uses half of another."""
            sl = slice(nt * 512, (nt + 1) * 512)
            steps = []

            def do_block_pair(blks, b=b):
                pt = pss_pool.tile([128, 1024], F32, tag="pss",
                                   name=f"qkv{b}_{nt}_{blks[0]}")
                for hh, blk in enumerate(blks):
                    ps = pt[:, hh * 512:(hh + 1) * 512]
                    csl = slice(blk * 128, (blk + 1) * 128)
                    for k in range(KS):
                        nc.tensor.matmul(ps, wqb[:, k, csl], xTb[b, k][:, sl],
                                         start=(k == 0), stop=False, **MM)
                    nc.tensor.matmul(ps, UW33[:, csl], SSD[b][:, sl],
                                     start=False, stop=True, **MM)
                    nc.vector.tensor_tensor(dsts[blk][b][:, sl], ps,
                                            rsig_bc[b][:, sl], op=AL.mult)
            steps.append(lambda: do_block_pair((0, 1)))
            steps.append(lambda: do_block_pair((2,)))
            return steps

        def emit_transpose(nt, b):
            def do_transpose():
                for t128 in range(4 * nt, 4 * nt + 4):
                    nc.sync.dma_start_transpose(
                        vA[b][:, t128, :],
                        vT[b][:, t128 * 128:(t128 + 1) * 128])
            return [do_transpose]

        for step in (emit_qkv_b(0, 0) + emit_qkv_b(1, 0)
                     + emit_transpose(0, 0) + emit_transpose(0, 1)):
            step()

        # ---- attention i-tiles, pipelined ----
        # Background work (out-proj of the previous tile, qkv + V-transpose of
        # the next slice) is drip-fed into each j-loop so the in-order PE
        # queue never parks behind a cross-engine dependency. P@V for step j
        # is emitted during step j+1 so the exp (ACT) -> bias-multiply (DVE)
        # chain of step j overlaps the score matmuls of step j+1.
        def emit_proj(t):
            steps = []
            for b in range(B):
                for t2 in range(4):
                    def do_proj(b=b, t2=t2, t=t):
                        tg = 4 * t + t2
                        psy = pss_pool.tile([128, 1024], F32, tag="pss",
                                            name=f"psy{b}_{tg}")
                        for half in range(2):
                            nc.tensor.matmul(
                                psy[:, half * 512:(half + 1) * 512],
                                oT[b][t][:, t2 * 128:(t2 + 1) * 128],
                                wob[:, half * 512:(half + 1) * 512],
                                start=True, stop=True, **MM)
                        yt = ysb.tile([128, D], F16, tag="yt")
                        nc.scalar.copy(yt[:], psy[:])
                        nc.gpsimd.dma_start(
                            y_out.ap()[b, tg * 128:(tg + 1) * 128, :], yt[:])
                    steps.append(do_proj)
            return steps

        def emit_lnrm(t, pso):
            for b in range(B):
                for h in range(HL):
                    lrow = lnrm.tile([1, 512], F32, tag="lrow")
                    nc.vector.tensor_copy(lrow[:], pso[(b, h)][64:65, :])
                    rec = lnrm.tile([1, 512], F32, tag="rec")
                    nc.vector.reciprocal_approx_fast(rec[:], lrow[:])
                    recb = lnrm.tile([1, 512], BF16, tag="recb")
                    nc.vector.tensor_copy(recb[:], rec[:])
                    lb = lnrm.tile([64, 512], BF16, tag="lb")
                    nc.gpsimd.partition_broadcast(lb[:], recb[:], channels=64)
                    nc.vector.tensor_tensor(
                        oT[b][t][h * 64:(h + 1) * 64, :],
                        pso[(b, h)][0:64, :], lb[:], op=AL.mult)

        bg = []
        for t in range(IT):
            nj = 4 * (t + 1)
            pso = {(b, h): pso_pool.tile([97, 512], F32, tag=f"pso{b}{h}",
                                         name=f"pso{b}{h}_{t}")
                   for b in range(B) for h in range(HL)}
            if t + 1 < IT:
                bg.extend(emit_qkv_b(0, t + 1) + emit_qkv_b(1, t + 1)
                          + emit_transpose(t + 1, 0) + emit_transpose(t + 1, 1))
            prev_pv = None
            for j in range(nj):
                # columns i < 128j of this i-slice are fully masked (causal trim)
                off = max(0, 128 * j - 512 * t)
                islo = slice(t * 512 + off, (t + 1) * 512)
                jt = slice(j * 128, (j + 1) * 128)
                if t == 0:
                    bt = bias_t0[j]
                else:
                    bt = bias_pool.tile([128, 1024], BF16, tag="bt")
                    isl_f = slice(t * 512, (t + 1) * 512)
                    nc.sync.dma_start(
                        bt[:].rearrange("p (h i) -> p h i", h=2),
                        ebias_in.ap()[0:2, jt, isl_f].rearrange("h j i -> j h i"))
                pebs = {}
                for b in range(B):
                    pss = pss_pool.tile([128, 1024], F32, tag="pss")
                    nc.tensor.matmul(pss[:, off:512], kT[b][0:64, jt],
                                     qT[b][0:64, islo],
                                     start=True, stop=True, **MM)
                    nc.tensor.matmul(pss[:, 512 + off:1024],
                                     kT[b][64:128, jt],
                                     qT[b][64:128, islo],
                                     start=True, stop=True, **MM)
                    pe = exp_pool.tile([128, 1024], BF16, tag="pe")
                    nc.scalar.activation(pe[:, off:], pss[:, off:],
                                         AF.Exp, bias=zeroc[:])
                    peb = exp_pool.tile([128, 1024], BF16, tag="peb")
                    nc.vector.tensor_tensor(peb[:, off:], pe[:, off:],
                                            bt[:, off:], op=AL.mult)
                    pebs[b] = peb
                if prev_pv is not None:
                    prev_pv()
                if bg:
                    bg.pop(0)()

                def make_pv(j=j, off=off, pebs=pebs):
                    def do_pv():
                        for b in range(B):
                            for h in range(HL):
                                nc.tensor.matmul(
                                    pso[(b, h)][0:64, off:],
                                    vA[b][:, j, h * 64:(h + 1) * 64],
                                    pebs[b][:, h * 512 + off:(h + 1) * 512],
                                    start=(j == 0), stop=(j == nj - 1), **MM)
                                # softmax denominator into partition 64
                                # (col group 2, concurrent with P@V)
                                nc.tensor.matmul(
                                    pso[(b, h)][64:65, off:], ones1[:],
                                    pebs[b][:, h * 512 + off:(h + 1) * 512],
                                    start=(j == 0), stop=(j == nj - 1),
                                    tile_position=(0, 64), **MM)
                    return do_pv
                prev_pv = make_pv()
            prev_pv()
            # drain remaining background work (the next slice's qkv must be
            # complete before the next tile's score matmuls read it)
            while bg:
                bg.pop(0)()
            emit_lnrm(t, pso)
            if t + 1 < IT:
                bg.extend(emit_proj(t))
        for step in emit_proj(IT - 1):
            step()

    nc.compile()
    return nc


_NC_CACHE = None


def _get_program():
    global _NC_CACHE
    if _NC_CACHE is None:
        _NC_CACHE = build_program()
    return _NC_CACHE


def build_in_maps(x, attn_bias, ln_gamma, ln_beta, w_qkv, w_out):
    x = np.asarray(x, dtype=np.float32)
    attn_bias = np.asarray(attn_bias, dtype=np.float32)
    ln_gamma = np.asarray(ln_gamma, dtype=np.float32)
    ln_beta = np.asarray(ln_beta, dtype=np.float32)
    w_qkv = np.asarray(w_qkv, dtype=np.float32)
    w_out = np.asarray(w_out, dtype=np.float32)

    xT = np.ascontiguousarray(x.transpose(0, 2, 1)).astype(ml_dtypes.bfloat16)
    # causal mask folded into exp(bias): zeros above the diagonal; transposed
    # to [head, key j, query i]
    tri = np.triu(np.ones((N, N), dtype=bool), k=1)  # True above diag (masked)
    in_maps = []
    for c in range(N_CORES):
        h0 = HL * c
        cols = np.concatenate([
            w_qkv[:, q * H * DH + h0 * DH: q * H * DH + (h0 + HL) * DH]
            for q in range(3)], axis=1)
        ebias = np.empty((HL, N, N), dtype=ml_dtypes.bfloat16)
        for h in range(HL):
            eb = np.exp(attn_bias[h0 + h])
            eb[tri] = 0.0
            ebias[h] = eb.T.astype(ml_dtypes.bfloat16)
        in_maps.append({
            "xT": xT,
            "ebias": ebias,
            "wqkv": np.ascontiguousarray(
                cols.reshape(KS, 128, COLS).transpose(1, 0, 2)),
            "wout": np.ascontiguousarray(w_out[h0 * DH:(h0 + HL) * DH]),
            "gamma": np.ascontiguousarray(ln_gamma.reshape(KS, 128).T),
            "beta": np.ascontiguousarray(ln_beta.reshape(KS, 128).T),
        })
    return in_maps


def kernel(x, attn_bias, ln_gamma, ln_beta, w_qkv, w_out):
    in_maps = build_in_maps(x, attn_bias, ln_gamma, ln_beta, w_qkv, w_out)
    nc = _get_program()
    res = run_bass_kernel_spmd(nc, in_maps, core_ids=list(range(N_CORES)))
    out = res.results[0]["y"].astype(np.float64)
    for c in range(1, N_CORES):
        out += res.results[c]["y"]
    return out.astype(np.float32)
